# revision 1
# baseline (speedup 1.0000x reference)
"""Trainium2 Bass kernel for nn_AreaPolyLoss (polygon rasterization MSE loss).

Sharding: data-parallel over batch B=8 -> one batch per NeuronCore (8 cores).
Each core:
  - gathers its K=32 vertex-offset vectors from `output` at `ind` locations
    (indirect DMA row-gather from a host-transposed [HW, C] layout),
  - builds the 512-vertex polygon (trunc(pred + centers)),
  - rasterizes via even-odd scanline counting: for scanline y and pixel x,
    inside = parity of #{edges: straddles y and x < xint}.  Using integer
    thresholds c = ceil(xint) masked by the straddle bit,
    count[y, x] = #{c > x}, computed by 128 fused compare+accumulate ops
    split across the Vector engine (tensor_scalar is_gt + accum) and the
    Scalar engine (Sign activation + accum).
  - reduces the masked MSE against `target` to a single partial sum.
Host combines the 8 per-core partials: loss = sum(mse)/16384 / (mask_sum*C+1e-4).
"""

import sys
import numpy as np

for _p in ("/opt/trn_rl_repo",):
    if _p not in sys.path:
        sys.path.insert(0, _p)

import concourse.bass as bass
import concourse.bacc as bacc
import concourse.mybir as mybir
from concourse import tile
from concourse.bass_utils import run_bass_kernel_spmd

B, K, C, H, W = 8, 32, 32, 128, 128
V = K * (C // 2)  # 512
HW = H * W

# x-loop split: DVE owns x in [0, ND), ACT owns x in [ND, 128)
ND = 74
NA = W - ND

F32 = mybir.dt.float32
I32 = mybir.dt.int32
ALU = mybir.AluOpType
ACTF = mybir.ActivationFunctionType

_cache = {}


def build_nc(nd=ND):
    na = W - nd
    nc = bacc.Bacc("TRN2", target_bir_lowering=False, debug=False, num_devices=8)

    outT = nc.dram_tensor("outT", [HW, C], F32, kind="ExternalInput")
    ind_d = nc.dram_tensor("ind", [K, 1], I32, kind="ExternalInput")
    cen_d = nc.dram_tensor("centers", [K, C], F32, kind="ExternalInput")
    msk_d = nc.dram_tensor("mask", [1, K], F32, kind="ExternalInput")
    tgt_d = nc.dram_tensor("target", [H, W], F32, kind="ExternalInput")
    out_d = nc.dram_tensor("out", [1, 2], F32, kind="ExternalOutput")

    with tile.TileContext(nc) as tc:
        with (
            tc.tile_pool(name="sbuf", bufs=1) as sp,
            tc.tile_pool(name="psum", bufs=1, space="PSUM") as pp,
            tc.tile_pool(name="dram", bufs=1, space="DRAM") as dp,
        ):
            # ---------- input DMAs first (the gather chain is critical) ----------
            ind_s = sp.tile([K, 1], I32, tag="ind_s")
            nc.sync.dma_start(ind_s[:], ind_d[:])
            cen_s = sp.tile([K, C], F32, tag="cen_s")
            nc.sync.dma_start(cen_s[:], cen_d[:])
            msk_s = sp.tile([1, K], F32, tag="msk_s")
            nc.sync.dma_start(msk_s[:], msk_d[:])
            tgt = sp.tile([H, W], F32, tag="tgt")
            nc.sync.dma_start(tgt[:], tgt_d[:])

            # gather pred[k, :] = outT[ind[k], :]
            pred = sp.tile([K, C], F32, tag="pred")
            nc.gpsimd.indirect_dma_start(
                out=pred[:], out_offset=None,
                in_=outT[:],
                in_offset=bass.IndirectOffsetOnAxis(ap=ind_s[:, 0:1], axis=0),
            )

            # ---------- constants ----------
            ones_col = sp.tile([128, 1], F32, tag="ones_col")
            nc.vector.memset(ones_col[:], 1.0)
            ones_row = sp.tile([1, 128], F32, tag="ones_row")
            nc.vector.memset(ones_row[:], 1.0)

            yi = sp.tile([128, 1], I32, tag="yi")
            nc.gpsimd.iota(yi[:], pattern=[[1, 1]], base=0, channel_multiplier=1)
            ycol = sp.tile([128, 1], F32, tag="ycol")
            nc.vector.tensor_copy(ycol[:], yi[:])

            # ACT bias columns: bias[:, i] = -(nd + i + 0.5)
            bi = sp.tile([128, na], I32, tag="bi")
            nc.gpsimd.iota(bi[:], pattern=[[-1, na]], base=0, channel_multiplier=0)
            bias_f = sp.tile([128, na], F32, tag="bias_f")
            nc.vector.tensor_copy(bias_f[:], bi[:])
            nc.vector.tensor_scalar(
                out=bias_f[:], in0=bias_f[:], scalar1=1.0, scalar2=-(nd + 0.5),
                op0=ALU.mult, op1=ALU.add)

            # warm up the Sign activation table ASAP
            sgn_warm = sp.tile([1, 1], F32, tag="sgn_warm")
            nc.scalar.activation(sgn_warm[:], ones_col[0:1, 0:1], ACTF.Sign)

            # ---------- vertices: trunc(pred + centers_expanded), [K, 32] ----------
            # host passes centers pre-expanded so x and y process in one chain.
            # trunc via the fp32 round-to-nearest magic trick (no mod on HW):
            #   rn = (x + M) - M  == round-to-nearest(x)   (exact for |x| < 2^22)
            #   trunc = rn - (rn > x) + (x < 0) * ((rn > x) + (rn < x))
            MAGIC = 12582912.0  # 1.5 * 2**23

            T = lambda nm: sp.tile([K, C], F32, tag=nm, name=nm)
            s = T("tr_s")
            nc.vector.tensor_tensor(out=s[:], in0=pred[:], in1=cen_s[:], op=ALU.add)
            rn = T("tr_rn")
            nc.vector.tensor_scalar(out=rn[:], in0=s[:], scalar1=MAGIC,
                                    scalar2=MAGIC, op0=ALU.add, op1=ALU.subtract)
            lt = T("tr_lt")
            nc.vector.scalar_tensor_tensor(out=lt[:], in0=rn[:], scalar=1.0,
                                           in1=s[:], op0=ALU.mult, op1=ALU.is_lt)
            gt = T("tr_gt")
            nc.vector.scalar_tensor_tensor(out=gt[:], in0=rn[:], scalar=1.0,
                                           in1=s[:], op0=ALU.mult, op1=ALU.is_gt)
            ng = T("tr_ng")
            nc.vector.tensor_scalar(out=ng[:], in0=s[:], scalar1=0.0,
                                    scalar2=None, op0=ALU.is_lt)
            gl = T("tr_gl")
            nc.vector.tensor_tensor(out=gl[:], in0=gt[:], in1=lt[:], op=ALU.add)
            ngl = T("tr_ngl")
            nc.vector.tensor_tensor(out=ngl[:], in0=ng[:], in1=gl[:], op=ALU.mult)
            t0 = T("tr_t0")
            nc.vector.tensor_tensor(out=t0[:], in0=rn[:], in1=gt[:], op=ALU.subtract)
            vt = T("vt")  # interleaved: x at even cols, y at odd cols
            nc.vector.tensor_tensor(out=vt[:], in0=t0[:], in1=ngl[:], op=ALU.add)
            vtr = vt[:].rearrange("k (j two) -> k j two", two=2)

            # ---------- flatten on-chip: SBUF -> SBUF [1, 2V] (no HBM hop) ----
            flat_s = sp.tile([1, 2 * V], F32, tag="flat_s")
            nc.sync.dma_start(
                flat_s[0:1, 0:V].rearrange("o (k j) -> o k j", k=K), vtr[:, :, 0])
            nc.sync.dma_start(
                flat_s[0:1, V:2 * V].rearrange("o (k j) -> o k j", k=K),
                vtr[:, :, 1])

            # ---------- broadcast via TensorE outer product, then to SBUF ----
            pxy = pp.tile([128, 2 * V], F32, tag="pxy")
            nc.tensor.matmul(pxy[:, 0:V], lhsT=ones_row[:], rhs=flat_s[0:1, 0:V],
                             start=True, stop=True)
            nc.tensor.matmul(pxy[:, V:2 * V], lhsT=ones_row[:],
                             rhs=flat_s[0:1, V:2 * V], start=True, stop=True)
            xy1g = sp.tile([128, 2 * V], F32, tag="xy1g")
            nc.vector.tensor_copy(xy1g[:], pxy[:])
            x1g = xy1g[:, 0:V]
            y1g = xy1g[:, V:2 * V]

            # ---------- per-(y, edge) grid math, [128, V] ----------
            G = lambda tag: sp.tile([128, V], F32, tag=tag, name=tag)
            dyg = G("dyg")
            nc.vector.tensor_tensor(out=dyg[:, 0:V - 1], in0=xy1g[:, V + 1:2 * V],
                                    in1=xy1g[:, V:2 * V - 1], op=ALU.subtract)
            nc.vector.tensor_tensor(out=dyg[:, V - 1:V], in0=xy1g[:, V:V + 1],
                                    in1=xy1g[:, 2 * V - 1:2 * V], op=ALU.subtract)
            dxg = G("dxg")
            nc.vector.tensor_tensor(out=dxg[:, 0:V - 1], in0=xy1g[:, 1:V],
                                    in1=xy1g[:, 0:V - 1], op=ALU.subtract)
            nc.vector.tensor_tensor(out=dxg[:, V - 1:V], in0=xy1g[:, 0:1],
                                    in1=xy1g[:, V - 1:V], op=ALU.subtract)
            # dy is integer-valued; +1e-30 only changes dy == 0 (avoids 1/0)
            dys = G("dys")
            nc.vector.tensor_scalar(out=dys[:], in0=dyg[:], scalar1=1e-30,
                                    scalar2=None, op0=ALU.add)
            inv = G("inv")
            nc.vector.reciprocal_approx_fast(out=inv[:], in_=dys[:])
            tq = G("tq")
            nc.vector.scalar_tensor_tensor(out=tq[:], in0=y1g, scalar=ycol[:, 0:1],
                                           in1=inv[:], op0=ALU.subtract, op1=ALU.mult)
            prod = G("prod")
            nc.vector.tensor_tensor(out=prod[:], in0=tq[:], in1=dxg[:], op=ALU.mult)
            xint = G("xint")
            nc.vector.tensor_tensor(out=xint[:], in0=x1g, in1=prod[:], op=ALU.subtract)
            g2 = G("g2")
            nc.vector.tensor_scalar(out=g2[:, 0:V - 1], in0=xy1g[:, V + 1:2 * V],
                                    scalar1=ycol[:, 0:1], scalar2=None, op0=ALU.is_le)
            nc.vector.tensor_scalar(out=g2[:, V - 1:V], in0=xy1g[:, V:V + 1],
                                    scalar1=ycol[:, 0:1], scalar2=None, op0=ALU.is_le)
            st = G("st")
            nc.vector.scalar_tensor_tensor(out=st[:], in0=y1g, scalar=ycol[:, 0:1],
                                           in1=g2[:], op0=ALU.is_le, op1=ALU.not_equal)
            # ceil(xint) = rn + (rn < xint), rn = round-to-nearest via magic
            rnx = G("rnx")
            nc.vector.tensor_scalar(out=rnx[:], in0=xint[:], scalar1=MAGIC,
                                    scalar2=MAGIC, op0=ALU.add, op1=ALU.subtract)
            ltx = G("ltx")
            nc.vector.scalar_tensor_tensor(out=ltx[:], in0=rnx[:], scalar=1.0,
                                           in1=xint[:], op0=ALU.mult, op1=ALU.is_lt)
            c1 = G("c1")
            nc.vector.tensor_tensor(out=c1[:], in0=rnx[:], in1=ltx[:], op=ALU.add)
            cth = G("cth")
            nc.vector.tensor_tensor(out=cth[:], in0=c1[:], in1=st[:], op=ALU.mult)

            # ---------- MSE weights (off critical chain) ----------
            wgt = sp.tile([H, W], F32, tag="wgt")
            nc.vector.tensor_scalar(out=wgt[:], in0=tgt[:], scalar1=-510.0,
                                    scalar2=65025.0, op0=ALU.mult, op1=ALU.add)
            colT = sp.tile([128, 1], F32, tag="colT")
            t2j = sp.tile([H, W], F32, tag="t2j")
            nc.vector.scalar_tensor_tensor(out=t2j[:], in0=tgt[:], scalar=1.0,
                                           in1=tgt[:], op0=ALU.mult, op1=ALU.mult,
                                           accum_out=colT[:])

            # ---------- x-loop: count[y, x] = #{cth > x} ----------
            countD = sp.tile([128, nd], F32, tag="countD")
            countA = sp.tile([128, na], F32, tag="countA")
            jd0 = G("jd0")
            jd1 = G("jd1")
            ja0 = G("ja0")
            ja1 = G("ja1")
            for x in range(nd):
                j = jd0 if x % 2 == 0 else jd1
                nc.vector.tensor_scalar(
                    out=j[:], in0=cth[:], scalar1=float(x), scalar2=0.0,
                    op0=ALU.is_gt, op1=ALU.add, accum_out=countD[:, x:x + 1])
            for i in range(na):
                j = ja0 if i % 2 == 0 else ja1
                nc.scalar.activation(
                    out=j[:], in_=cth[:], func=ACTF.Sign,
                    bias=bias_f[:, i:i + 1], scale=1.0,
                    accum_out=countA[:, i:i + 1])

            # ---------- parity + masked MSE ----------
            # parity(n) = n - 2*round_nearest(0.5*n - 0.25), exact for int n >= 0
            def parity(out_t, cnt_ap, wid, tagp):
                h = sp.tile([128, wid], F32, tag=tagp + "_h", name=tagp + "_h")
                nc.vector.tensor_scalar(out=h[:], in0=cnt_ap, scalar1=0.5,
                                        scalar2=-0.25, op0=ALU.mult, op1=ALU.add)
                fl = sp.tile([128, wid], F32, tag=tagp + "_fl", name=tagp + "_fl")
                nc.vector.tensor_scalar(out=fl[:], in0=h[:], scalar1=MAGIC,
                                        scalar2=MAGIC, op0=ALU.add, op1=ALU.subtract)
                nc.vector.scalar_tensor_tensor(out=out_t[:], in0=fl[:], scalar=-2.0,
                                               in1=cnt_ap, op0=ALU.mult, op1=ALU.add)

            insD = sp.tile([128, nd], F32, tag="insD")
            parity(insD, countD[:], nd, "pD")
            cAf = sp.tile([128, na], F32, tag="cAf")
            nc.scalar.activation(out=cAf[:], in_=countA[:], func=ACTF.Copy,
                                 scale=0.5, bias=256.0)
            insA = sp.tile([128, na], F32, tag="insA")
            parity(insA, cAf[:], na, "pA")

            colD = sp.tile([128, 1], F32, tag="colD")
            colA = sp.tile([128, 1], F32, tag="colA")
            cDj = sp.tile([128, nd], F32, tag="cDj")
            cAj = sp.tile([128, na], F32, tag="cAj")
            nc.vector.scalar_tensor_tensor(out=cDj[:], in0=insD[:], scalar=1.0,
                                           in1=wgt[:, 0:nd], op0=ALU.mult,
                                           op1=ALU.mult, accum_out=colD[:])
            nc.vector.scalar_tensor_tensor(out=cAj[:], in0=insA[:], scalar=1.0,
                                           in1=wgt[:, nd:W], op0=ALU.mult,
                                           op1=ALU.mult, accum_out=colA[:])

            rows = sp.tile([128, 1], F32, tag="rows")
            nc.vector.tensor_tensor(out=rows[:], in0=colD[:], in1=colA[:], op=ALU.add)
            nc.vector.tensor_tensor(out=rows[:], in0=rows[:], in1=colT[:], op=ALU.add)

            psr = pp.tile([1, 1], F32, tag="psr")
            nc.tensor.matmul(psr[:], lhsT=rows[:, 0:1], rhs=ones_col[:, 0:1],
                             start=True, stop=True)

            mj = sp.tile([1, K], F32, tag="mj")
            msum = sp.tile([1, 1], F32, tag="msum")
            nc.vector.tensor_scalar(out=mj[:], in0=msk_s[:], scalar1=1.0, scalar2=0.0,
                                    op0=ALU.mult, op1=ALU.add, accum_out=msum[:])

            outt = sp.tile([1, 2], F32, tag="outt")
            nc.vector.tensor_copy(outt[:, 0:1], psr[:])
            nc.vector.tensor_copy(outt[:, 1:2], msum[:])
            nc.sync.dma_start(out_d[:], outt[:])

    nc.compile()
    return nc


def _shard_inputs(output, mask, ind, target, centers):
    in_maps = []
    for b in range(B):
        outT = np.ascontiguousarray(
            output[b].reshape(C, HW).T).astype(np.float32, copy=False)
        in_maps.append({
            "outT": outT,
            "ind": np.ascontiguousarray(ind[b].reshape(K, 1)).astype(np.int32, copy=False),
            "centers": np.ascontiguousarray(np.tile(centers[b].astype(np.float32), (1, C // 2))),
            "mask": np.ascontiguousarray(mask[b].reshape(1, K)).astype(np.float32, copy=False),
            "target": np.ascontiguousarray(target[b]).astype(np.float32, copy=False),
        })
    return in_maps


def _combine(results):
    pix = np.float32(0.0)
    msk = np.float32(0.0)
    for r in results:
        o = r["out"].reshape(2).astype(np.float32)
        pix += o[0]
        msk += o[1]
    mse_sum = pix / np.float32(HW)
    denom = msk * np.float32(C) + np.float32(1e-4)
    return np.float32(mse_sum / denom)


def run(inputs, trace=False, **kw):
    if "nc" not in _cache:
        _cache["nc"] = build_nc()
    nc = _cache["nc"]
    in_maps = _shard_inputs(**inputs)
    res = run_bass_kernel_spmd(nc, in_maps, core_ids=list(range(8)), trace=trace, **kw)
    return _combine(res.results), res


def kernel(output, mask, ind, target, centers):
    inputs = {
        "output": np.asarray(output, dtype=np.float32),
        "mask": np.asarray(mask, dtype=np.float32),
        "ind": np.asarray(ind, dtype=np.int32),
        "target": np.asarray(target, dtype=np.float32),
        "centers": np.asarray(centers, dtype=np.float32),
    }
    loss, _ = run(inputs)
    return loss

